# revision 35
# baseline (speedup 1.0000x reference)
"""Trainium2 Bass kernel for nn_MentionScore (span embeddings + mention scores).

Data-parallel over the batch dim: 8 documents -> 8 NeuronCores, one SPMD NEFF.
All tensor math runs on-device; host only prepares index metadata (padded
start/end rows as f32 and per-span-tile k-window bounds, recomputed per call).
"""
import numpy as np

import concourse.bacc as bacc
import concourse.bass as bass
import concourse.tile as tile
from concourse import mybir
from concourse.bass_utils import run_bass_kernel_spmd
from concourse.masks import make_identity

F32 = mybir.dt.float32
OP = mybir.AluOpType
ACTF = mybir.ActivationFunctionType

B, T, E, S = 8, 2048, 1024, 2000
H = 150
NT, NE, NS = T // 128, E // 128, 2048 // 128  # 16, 8, 16
SPAD = 2048


def build_nc(meta):
    nc = bacc.Bacc("TRN2", target_bir_lowering=False, debug=False, num_devices=8)

    x_d = nc.dram_tensor("x", [T, E], F32, kind="ExternalInput")
    wcat_d = nc.dram_tensor("wcat", [E, 600], F32, kind="ExternalInput")
    aw2_d = nc.dram_tensor("aw2", [H, H], F32, kind="ExternalInput")
    aw3_d = nc.dram_tensor("aw3", [H, 1], F32, kind="ExternalInput")
    ab1_d = nc.dram_tensor("ab1", [1, H], F32, kind="ExternalInput")
    ab2_d = nc.dram_tensor("ab2", [H, 1], F32, kind="ExternalInput")
    sw2_d = nc.dram_tensor("sw2", [H, H], F32, kind="ExternalInput")
    sw3_d = nc.dram_tensor("sw3", [H, 1], F32, kind="ExternalInput")
    sb1_d = nc.dram_tensor("sb1", [1, H], F32, kind="ExternalInput")
    sb2_d = nc.dram_tensor("sb2", [H, 1], F32, kind="ExternalInput")
    stf_d = nc.dram_tensor("startsf", [1, SPAD], F32, kind="ExternalInput")
    enf_d = nc.dram_tensor("endsf", [1, SPAD], F32, kind="ExternalInput")
    sti_d = nc.dram_tensor("startsi", [128, NS], mybir.dt.int32, kind="ExternalInput")
    eni_d = nc.dram_tensor("endsi", [128, NS], mybir.dt.int32, kind="ExternalInput")

    se_d = nc.dram_tensor("se", [SPAD, 3 * E], F32, kind="ExternalOutput")
    sc_d = nc.dram_tensor("scores", [1, SPAD], F32, kind="ExternalOutput")

    ab3 = float(meta["a_b3"])
    sb3 = float(meta["s_b3"])
    kwin = meta["kwin"]  # per s_tile: (s_lo, s_hi, e_lo, e_hi) inclusive k bounds

    with tile.TileContext(nc) as tc:
        with (
            tc.tile_pool(name="const", bufs=1) as cp,
            tc.tile_pool(name="h1s", bufs=NS) as h1sp,
            tc.tile_pool(name="sestage", bufs=2) as sep,
            tc.tile_pool(name="work", bufs=2) as wk,
            tc.tile_pool(name="chunk", bufs=1) as wkb,
        ):
            # ---- constants / weights in SBUF ----
            ident = cp.tile([128, 128], F32, tag="ident")
            make_identity(nc, ident[:])

            wcat_sb = []
            for e in range(NE):
                w = cp.tile([128, 600], F32, tag=f"wcat{e}")
                nc.sync.dma_start(w[:], wcat_d[e * 128:(e + 1) * 128, :])
                wcat_sb.append(w)

            def load2(dram, cols, tagp):
                t0 = cp.tile([128, cols], F32, tag=f"{tagp}0")
                t1 = cp.tile([22, cols], F32, tag=f"{tagp}1")
                nc.sync.dma_start(t0[:], dram[0:128, :])
                nc.sync.dma_start(t1[:], dram[128:H, :])
                return t0, t1

            aw2_sb = load2(aw2_d, H, "aw2")
            sw2_sb = load2(sw2_d, H, "sw2")
            aw3_sb = load2(aw3_d, 1, "aw3")
            sw3_sb = load2(sw3_d, 1, "sw3")
            ab2_sb = load2(ab2_d, 1, "ab2")
            sb2_sb = load2(sb2_d, 1, "sb2")

            ab1_bc = cp.tile([128, H], F32, tag="ab1bc")
            nc.sync.dma_start(ab1_bc[:], ab1_d[0:1, :].to_broadcast((128, H)))
            sb1_bc = cp.tile([128, H], F32, tag="sb1bc")
            nc.sync.dma_start(sb1_bc[:], sb1_d[0:1, :].to_broadcast((128, H)))

            sti_sb = cp.tile([128, NS], mybir.dt.int32, tag="stisb")
            nc.sync.dma_start(sti_sb[:], sti_d[:, :])
            eni_sb = cp.tile([128, NS], mybir.dt.int32, tag="enisb")
            nc.sync.dma_start(eni_sb[:], eni_d[:, :])

            iota_i = cp.tile([128, NT], mybir.dt.int32, tag="iotai")
            nc.gpsimd.iota(iota_i[:], pattern=[[128, NT]], base=0, channel_multiplier=1)
            iota_f = cp.tile([128, NT], F32, tag="iotaf")
            nc.vector.tensor_copy(iota_f[:], iota_i[:])

            attns = cp.tile([128, NT], F32, tag="attns")

            with tc.tile_pool(name="big", bufs=NT) as bigp, \
                 tc.tile_pool(name="dscratch", bufs=1, space="DRAM") as dsp:
                p1_d = dsp.tile([T, H], F32, tag="p1d")
                p2_d = dsp.tile([T, H], F32, tag="p2d")
                x_sb = []
                for t in range(NT):
                    xt = bigp.tile([128, E], F32, tag="x")
                    nc.sync.dma_start(xt[:], x_d[t * 128:(t + 1) * 128, :])
                    x_sb.append(xt)
                p123_sb = [bigp.tile([128, H], F32, name=f"p123_{i}", tag="p123") for i in range(NT)]
                h1a_sb = [bigp.tile([128, H], F32, name=f"h1a_{i}", tag="h1a") for i in range(NT)]

                # ---- stage BC: X^T tiles on the fly; H_all = X @ wcat ----
                with tc.tile_pool(name="ps_bc", bufs=2, space="PSUM") as pbc:
                    for t in range(NT):
                        ps512 = pbc.tile([128, 512], F32, tag="hall512")
                        ps88 = pbc.tile([128, 88], F32, tag="hall88")
                        for e in range(NE):
                            xtp = pbc.tile([128, 128], F32, tag="xtp")
                            nc.tensor.transpose(
                                out=xtp[:], in_=x_sb[t][:, e * 128:(e + 1) * 128],
                                identity=ident[:])
                            xts = wk.tile([128, 128], F32, tag="xts")
                            if (t * NE + e) % 2 == 0:
                                nc.vector.tensor_copy(xts[:], xtp[:])
                            else:
                                nc.scalar.copy(xts[:], xtp[:])
                            nc.tensor.matmul(ps512[:], lhsT=xts[:],
                                             rhs=wcat_sb[e][:, 0:512],
                                             start=(e == 0), stop=(e == NE - 1))
                            nc.tensor.matmul(ps88[:], lhsT=xts[:],
                                             rhs=wcat_sb[e][:, 512:600],
                                             start=(e == 0), stop=(e == NE - 1))
                        # evac: h1a (bias+relu) and P1|P2|P3 raw
                        nc.vector.tensor_tensor(out=h1a_sb[t][:], in0=ps512[:, 0:H],
                                                in1=ab1_bc[:], op=OP.add)
                        nc.vector.tensor_scalar_max(h1a_sb[t][:], h1a_sb[t][:], 0.0)
                        p1_t = wk.tile([128, H], F32, tag="p1t")
                        nc.scalar.copy(p1_t[:], ps512[:, 150:300])
                        nc.sync.dma_start(p1_d[t * 128:(t + 1) * 128, :], p1_t[:])
                        p2_t = wk.tile([128, H], F32, tag="p2t")
                        nc.scalar.copy(p2_t[:], ps512[:, 300:450])
                        nc.sync.dma_start(p2_d[t * 128:(t + 1) * 128, :], p2_t[:])
                        nc.scalar.copy(p123_sb[t][:, 0:62], ps512[:, 450:512])
                        nc.scalar.copy(p123_sb[t][:, 62:150], ps88[:, 0:88])
                        # start_e / end_e rows for span tile st=t: pure DMA pipeline,
                        # no PE dependency -> overlaps the matmul phases
                        gt = sep.tile([128, 2 * E], F32, tag="gt")
                        nc.gpsimd.indirect_dma_start(
                            out=gt[:, 0:E], out_offset=None, in_=x_d[:, :],
                            in_offset=bass.IndirectOffsetOnAxis(ap=sti_sb[:, t:t + 1], axis=0))
                        nc.gpsimd.indirect_dma_start(
                            out=gt[:, E:2 * E], out_offset=None, in_=x_d[:, :],
                            in_offset=bass.IndirectOffsetOnAxis(ap=eni_sb[:, t:t + 1], axis=0))
                        nc.sync.dma_start(se_d[t * 128:(t + 1) * 128, 0:2 * E], gt[:])

                # ---- MLP_a layers 2/3 (streamed per 512-token chunk) ----
                with tc.tile_pool(name="ps_a", bufs=1, space="PSUM") as pa, \
                     tc.tile_pool(name="ps_atr", bufs=2, space="PSUM") as patr:
                    for c in range(4):
                        h1aT0 = wkb.tile([128, 512], F32, tag="h1aT0")
                        h1aT1 = wkb.tile([22, 512], F32, tag="h1aT1")
                        for tt in range(4):
                            t = c * 4 + tt
                            tp0 = patr.tile([128, 128], F32, tag="atr0")
                            nc.tensor.transpose(out=tp0[:], in_=h1a_sb[t][:, 0:128],
                                                identity=ident[:])
                            if tt % 2 == 0:
                                nc.vector.tensor_copy(h1aT0[:, tt * 128:(tt + 1) * 128], tp0[:])
                            else:
                                nc.scalar.copy(h1aT0[:, tt * 128:(tt + 1) * 128], tp0[:])
                            tp1 = patr.tile([22, 128], F32, tag="atr1")
                            nc.tensor.transpose(out=tp1[:], in_=h1a_sb[t][:, 128:H],
                                                identity=ident[:])
                            if tt % 2 == 0:
                                nc.scalar.copy(h1aT1[:, tt * 128:(tt + 1) * 128], tp1[:])
                            else:
                                nc.vector.tensor_copy(h1aT1[:, tt * 128:(tt + 1) * 128], tp1[:])
                        h2aT = []
                        for hc, hsz in ((0, 128), (1, 22)):
                            l2 = pa.tile([hsz, 512], F32, tag=f"al2_{hc}")
                            nc.tensor.matmul(l2[:], lhsT=aw2_sb[0][:, hc * 128:hc * 128 + hsz],
                                             rhs=h1aT0[:], start=True, stop=False)
                            nc.tensor.matmul(l2[:], lhsT=aw2_sb[1][:, hc * 128:hc * 128 + hsz],
                                             rhs=h1aT1[:], start=False, stop=True)
                            h2 = wkb.tile([hsz, 512], F32, tag=f"h2aT{hc}")
                            nc.scalar.activation(h2[:], l2[:], ACTF.Relu,
                                                 bias=ab2_sb[hc][0:hsz, 0:1])
                            h2aT.append(h2)
                        for tt in range(4):
                            t = c * 4 + tt
                            ap3 = pa.tile([128, 1], F32, tag="al3")
                            nc.tensor.matmul(ap3[:], lhsT=h2aT[0][:, tt * 128:(tt + 1) * 128],
                                             rhs=aw3_sb[0][:], start=True, stop=False)
                            nc.tensor.matmul(ap3[:], lhsT=h2aT[1][:, tt * 128:(tt + 1) * 128],
                                             rhs=aw3_sb[1][:22, :], start=False, stop=True)
                            nc.vector.tensor_scalar_add(attns[:, t:t + 1], ap3[:], ab3)

                # ---- stage E: per span-tile selection matmuls ----
                with tc.tile_pool(name="ps_e", bufs=2, space="PSUM") as pe, \
                     tc.tile_pool(name="ps_h", bufs=2, space="PSUM") as ph:
                    h1s_sb = []
                    for st in range(NS):
                        s_lo, s_hi, e_lo, e_hi = kwin[st]
                        strow = wk.tile([1, 128], F32, tag="strow")
                        nc.sync.dma_start(strow[:], stf_d[0:1, st * 128:(st + 1) * 128])
                        enrow = wk.tile([1, 128], F32, tag="enrow")
                        nc.sync.dma_start(enrow[:], enf_d[0:1, st * 128:(st + 1) * 128])
                        bst = wk.tile([128, 128], F32, tag="bst")
                        nc.gpsimd.partition_broadcast(bst[:], strow[0:1, :])
                        ben = wk.tile([128, 128], F32, tag="ben")
                        nc.gpsimd.partition_broadcast(ben[:], enrow[0:1, :])
                        se_t = sep.tile([128, E], F32, tag="se")
                        g1 = wk.tile([128, H], F32, tag="g1")
                        nc.gpsimd.indirect_dma_start(
                            out=g1[:], out_offset=None, in_=p1_d[:, :],
                            in_offset=bass.IndirectOffsetOnAxis(ap=sti_sb[:, st:st + 1], axis=0))
                        g2 = wk.tile([128, H], F32, tag="g2")
                        nc.gpsimd.indirect_dma_start(
                            out=g2[:], out_offset=None, in_=p2_d[:, :],
                            in_offset=bass.IndirectOffsetOnAxis(ap=eni_sb[:, st:st + 1], axis=0))
                        accs = [pe.tile([128, 512], F32, name=f"acc_t{i}", tag=f"acc{i}") for i in range(2)]
                        h1ps = ph.tile([128, H], F32, tag="h1ps")
                        n_mm = e_hi - s_lo + 1
                        mm_i = 0
                        for k in range(s_lo, e_hi + 1):
                            tval = iota_f[:, k:k + 1].to_broadcast((128, 128))
                            ge = wk.tile([128, 128], F32, tag="ge")
                            nc.vector.tensor_tensor(out=ge[:], in0=tval, in1=bst[:], op=OP.is_ge)
                            le = wk.tile([128, 128], F32, tag="le")
                            nc.vector.tensor_tensor(out=le[:], in0=tval, in1=ben[:], op=OP.is_le)
                            selr = wk.tile([128, 128], F32, tag="selr")
                            # (ge * attns) * le: scaled range mask in one op
                            nc.vector.scalar_tensor_tensor(
                                out=selr[:], in0=ge[:], scalar=attns[:, k:k + 1],
                                in1=le[:], op0=OP.mult, op1=OP.mult)
                            nc.tensor.matmul(accs[0][:], lhsT=selr[:], rhs=x_sb[k][:, 0:512],
                                             start=(k == s_lo), stop=(k == e_hi))
                            nc.tensor.matmul(accs[1][:], lhsT=selr[:], rhs=x_sb[k][:, 512:E],
                                             start=(k == s_lo), stop=(k == e_hi))
                            nc.tensor.matmul(h1ps[:], lhsT=selr[:], rhs=p123_sb[k][:, 0:H],
                                             start=(mm_i == 0), stop=(mm_i == n_mm - 1))
                            mm_i += 1
                        # evacuate span sums + h1s
                        nc.vector.tensor_copy(se_t[:, 0:512], accs[0][:])
                        nc.scalar.copy(se_t[:, 512:1024], accs[1][:])
                        h1s = h1sp.tile([128, H], F32, tag="h1s")
                        nc.vector.tensor_tensor(out=h1s[:], in0=g1[:], in1=g2[:], op=OP.add)
                        nc.vector.tensor_tensor(out=h1s[:], in0=h1s[:], in1=h1ps[:], op=OP.add)
                        nc.vector.tensor_tensor(out=h1s[:], in0=h1s[:], in1=sb1_bc[:], op=OP.add)
                        nc.vector.tensor_scalar_max(h1s[:], h1s[:], 0.0)
                        h1s_sb.append(h1s)
                        nc.sync.dma_start(se_d[st * 128:(st + 1) * 128, 2 * E:3 * E], se_t[:])

            # ---- MLP_s layers 2/3 (big pool closed; streamed per 512 spans) ----
            with tc.tile_pool(name="ps_s", bufs=1, space="PSUM") as psm, \
                 tc.tile_pool(name="ps_str", bufs=2, space="PSUM") as pstr:
                for c in range(4):
                    h1sT0 = wkb.tile([128, 512], F32, tag="h1sT0")
                    h1sT1 = wkb.tile([22, 512], F32, tag="h1sT1")
                    for tt in range(4):
                        st = c * 4 + tt
                        tp0 = pstr.tile([128, 128], F32, tag="str0")
                        nc.tensor.transpose(out=tp0[:], in_=h1s_sb[st][:, 0:128],
                                            identity=ident[:])
                        if tt % 2 == 0:
                            nc.vector.tensor_copy(h1sT0[:, tt * 128:(tt + 1) * 128], tp0[:])
                        else:
                            nc.scalar.copy(h1sT0[:, tt * 128:(tt + 1) * 128], tp0[:])
                        tp1 = pstr.tile([22, 128], F32, tag="str1")
                        nc.tensor.transpose(out=tp1[:], in_=h1s_sb[st][:, 128:H],
                                            identity=ident[:])
                        if tt % 2 == 0:
                            nc.scalar.copy(h1sT1[:, tt * 128:(tt + 1) * 128], tp1[:])
                        else:
                            nc.vector.tensor_copy(h1sT1[:, tt * 128:(tt + 1) * 128], tp1[:])
                    h2sT = []
                    for hc, hsz in ((0, 128), (1, 22)):
                        l2 = psm.tile([hsz, 512], F32, tag=f"sl2_{hc}")
                        nc.tensor.matmul(l2[:], lhsT=sw2_sb[0][:, hc * 128:hc * 128 + hsz],
                                         rhs=h1sT0[:], start=True, stop=False)
                        nc.tensor.matmul(l2[:], lhsT=sw2_sb[1][:, hc * 128:hc * 128 + hsz],
                                         rhs=h1sT1[:], start=False, stop=True)
                        h2 = wkb.tile([hsz, 512], F32, tag=f"h2sT{hc}")
                        nc.scalar.activation(h2[:], l2[:], ACTF.Relu,
                                             bias=sb2_sb[hc][0:hsz, 0:1])
                        h2sT.append(h2)
                    l3 = psm.tile([1, 512], F32, tag="sl3")
                    nc.tensor.matmul(l3[:], lhsT=sw3_sb[0][:], rhs=h2sT[0][:],
                                     start=True, stop=False)
                    nc.tensor.matmul(l3[:], lhsT=sw3_sb[1][:22, :], rhs=h2sT[1][:],
                                     start=False, stop=True)
                    sc_t = wk.tile([1, 512], F32, tag="sct")
                    nc.vector.tensor_scalar_add(sc_t[0:1, :], l3[0:1, :], sb3)
                    nc.sync.dma_start(sc_d[0:1, c * 512:(c + 1) * 512], sc_t[:])

    nc.compile()
    return nc


LAST_EXEC_NS = None
LAST_NC = None


def kernel(**inputs):
    inp = {k: np.asarray(v) for k, v in inputs.items()}
    x = inp["batch_embeds"].astype(np.float32)          # [B, T, E]
    starts = inp["span_starts"].astype(np.int64)        # [B, S]
    lengths = inp["span_lengths"].astype(np.int64)      # [B, S]
    ends = starts + lengths

    wcat = np.concatenate(
        [inp["a_w1"], inp["s_w1"][0:E], inp["s_w1"][E:2 * E], inp["s_w1"][2 * E:3 * E]],
        axis=1).astype(np.float32)                      # [E, 600]

    PADV = -1.0e6
    stf = np.full((B, 1, SPAD), PADV, np.float32)
    enf = np.full((B, 1, SPAD), PADV, np.float32)
    stf[:, 0, :S] = starts.astype(np.float32)
    enf[:, 0, :S] = ends.astype(np.float32)
    # int32 gather indices, [128, NS] (partition-major per span tile), pads -> row 0
    sti = np.zeros((B, SPAD), np.int32)
    eni = np.zeros((B, SPAD), np.int32)
    sti[:, :S] = starts.astype(np.int32)
    eni[:, :S] = ends.astype(np.int32)
    sti = sti.reshape(B, NS, 128).transpose(0, 2, 1)
    eni = eni.reshape(B, NS, 128).transpose(0, 2, 1)

    # k-tile windows per span-tile (union across batches; one SPMD NEFF)
    kwin = []
    for st in range(NS):
        lo = st * 128
        hi = min((st + 1) * 128, S)
        if lo >= S:
            kwin.append(kwin[-1])
            continue
        ss = starts[:, lo:hi]
        ee = ends[:, lo:hi]
        s_lo, s_hi = int(ss.min()) // 128, int(ss.max()) // 128
        e_lo, e_hi = int(ee.min()) // 128, int(ee.max()) // 128
        kwin.append((s_lo, s_hi, e_lo, e_hi))

    meta = {
        "a_b3": float(np.asarray(inp["a_b3"]).reshape(-1)[0]),
        "s_b3": float(np.asarray(inp["s_b3"]).reshape(-1)[0]),
        "kwin": kwin,
    }
    nc = build_nc(meta)
    global LAST_NC
    LAST_NC = nc

    common = {
        "wcat": wcat,
        "aw2": inp["a_w2"].astype(np.float32),
        "aw3": inp["a_w3"].astype(np.float32),
        "ab1": inp["a_b1"].reshape(1, H).astype(np.float32),
        "ab2": inp["a_b2"].reshape(H, 1).astype(np.float32),
        "sw2": inp["s_w2"].astype(np.float32),
        "sw3": inp["s_w3"].astype(np.float32),
        "sb1": inp["s_b1"].reshape(1, H).astype(np.float32),
        "sb2": inp["s_b2"].reshape(H, 1).astype(np.float32),
    }
    in_maps = []
    for b in range(B):
        m = dict(common)
        m["x"] = np.ascontiguousarray(x[b])
        m["startsf"] = np.ascontiguousarray(stf[b])
        m["endsf"] = np.ascontiguousarray(enf[b])
        m["startsi"] = np.ascontiguousarray(sti[b])
        m["endsi"] = np.ascontiguousarray(eni[b])
        in_maps.append(m)

    import os
    try:
        res = run_bass_kernel_spmd(nc, in_maps, core_ids=list(range(8)))
    except ModuleNotFoundError:
        # axon NTFF profiling hook unavailable in this container; run untraced
        os.environ["BASS_NEVER_TRACE"] = "1"
        res = run_bass_kernel_spmd(nc, in_maps, core_ids=list(range(8)))
    global LAST_EXEC_NS
    LAST_EXEC_NS = res.exec_time_ns

    se = np.stack([res.results[b]["se"][:S] for b in range(B)])
    ms = np.stack([res.results[b]["scores"][0, :S, None] for b in range(B)])
    return se.astype(np.float32), ms.astype(np.float32)
